# revision 44
# baseline (speedup 1.0000x reference)
"""AdaAtt attention block on 8 TRN2 NeuronCores.

Sharding: data-parallel over batch (16 batches/core), weights replicated.
All dense-layer activations are kept in transposed layout [D_part, batch]
so every D x D matmul consumes host-pre-transposed weights with natural
DMA, and bias+nonlinearity fuse into one ScalarE op out of PSUM.

The conv streams move as one [98, 2, 1024] DMA per batch (two 98-row
l-chunks) — multiple 4KB rows per partition makes the descriptors fan
across SDMA engines (~205 GB/s vs 33 GB/s for single-row-per-partition).
The fake-region slot (l=0) is handled by a separate batched [16, 1024]
pipeline instead of per-batch row injection.

Score pass per (batch, chunk): DVE add (ho_e broadcast via PE outer
product into PSUM) -> ACT tanh -> DVE scalar_tensor_tensor (x*W_a fused
with the free-axis sum) -> score columns. One batched softmax for all 16
batches (b_a dropped: softmax is shift-invariant). visAtt accumulates
into [d, batch] PSUM via per-chunk matvecs; the l=0 term joins through
PE transposes of PI0*fr. Then +ho_T and the final tanh(W_h @ .) matmul.
"""

import sys

if "/opt/trn_rl_repo" not in sys.path:
    sys.path.insert(0, "/opt/trn_rl_repo")

import numpy as np

import concourse.bass as bass
import concourse.tile as tile
from concourse import mybir
from concourse import bacc
from concourse.bass_utils import run_bass_kernel_spmd
from concourse.masks import make_identity

# ---------------------------------------------------------------------------

B, L, D = 128, 196, 1024
N_CORES = 8
S = B // N_CORES          # batches per core
CH = D // 128             # 128-wide chunks of D
LC = L // 2               # l-chunk rows (98); l=0 slot handled separately
NEG = -1.0e30

F32 = mybir.dt.float32
BF16 = mybir.dt.bfloat16
STREAM_DT = BF16          # dtype of conv_feat/conv_feat_embed stream
WEIGHT_DT = BF16          # dtype of the five D x D weights / dense math

ACTF = mybir.ActivationFunctionType
ALU = mybir.AluOpType


def _np_dt(dt):
    if dt == F32:
        return np.float32
    import ml_dtypes

    return ml_dtypes.bfloat16


def build_nc(stage: int = 9) -> bass.Bass:
    # stage: debug bisection knob — 0: dense chains, 1: +naturalize,
    # 2: +score pass, 3: +softmax, 4+: full kernel.
    nc = bacc.Bacc()

    def param(name, shape, dt=F32, out=False):
        return nc.declare_dram_parameter(name, list(shape), dt, isOutput=out)

    fr_in_T = param("fake_region_T", (D, S), WEIGHT_DT)
    ho_in_T = param("h_out_T", (D, S), WEIGHT_DT)
    w_names = ("W_fr_T", "W_fre_T", "W_ho_T", "W_hoe_T", "W_h_T")
    w_dram = {n: param(n, (D, D), WEIGHT_DT) for n in w_names}
    b_names = ("b_fr", "b_fre", "b_ho", "b_hoe", "b_h")
    b_dram = {n: param(n, (D,)) for n in b_names}
    wa_dram = param("W_a", (D,), STREAM_DT)
    conv = param("conv_feat", (S, L, D), STREAM_DT)
    cfe = param("conv_feat_embed", (S, L, D), STREAM_DT)
    out_dram = param("out", (D, S), out=True)

    with tile.TileContext(nc) as tc:
        with (
            tc.tile_pool(name="singles", bufs=1) as singles,
            tc.tile_pool(name="wpool", bufs=2) as wpool,
            tc.tile_pool(name="acts", bufs=1) as acts,
            tc.tile_pool(name="stream", bufs=3) as stream,
            tc.tile_pool(name="convp", bufs=S) as convp,
            tc.tile_pool(name="bcast", bufs=2) as bcast,
            tc.tile_pool(name="mm_psum", bufs=2, space="PSUM") as mm_psum,
            tc.tile_pool(name="tr_psum", bufs=2, space="PSUM") as tr_psum,
            tc.tile_pool(name="va_psum", bufs=1, space="PSUM") as va_psum,
        ):
            # --- constants -------------------------------------------------
            _idents = {}

            def get_ident(dt):
                if dt not in _idents:
                    t = singles.tile([128, 128], dt, tag=f"ident_{dt}")
                    make_identity(nc, t)
                    _idents[dt] = t
                return _idents[dt]

            wa_bc = singles.tile([128, 2, D], STREAM_DT)
            wa_ap = wa_dram[:]
            nc.sync.dma_start(
                out=wa_bc,
                in_=bass.AP(
                    tensor=wa_ap.tensor, offset=wa_ap.offset,
                    ap=[[0, 128], [0, 2]] + list(wa_ap.ap),
                ),
            )

            bias_sb = {}
            for n in b_names:
                t = singles.tile([128, CH], F32, tag=f"bias_{n}")
                nc.sync.dma_start(
                    out=t, in_=b_dram[n][:].rearrange("(c p) -> p c", p=128)
                )
                bias_sb[n] = t

            # --- stage A: fr/fr_e/ho/ho_e in transposed layout -------------
            def load_acts(src):
                t = acts.tile([128, CH, S], WEIGHT_DT, tag="acts_in")
                nc.sync.dma_start(
                    out=t, in_=src[:, :].rearrange("(c p) b -> p c b", p=128)
                )
                return t

            def dense_T(w_name, b_name, rhs_sb, func, out_tag, out_dt=None):
                """out[128, CH, S] = func(W^T . rhs + b), transposed layout."""
                w_sb = wpool.tile([128, CH, D], WEIGHT_DT, tag="w")
                nc.sync.dma_start(
                    out=w_sb,
                    in_=w_dram[w_name][:, :].rearrange("(kc kp) o -> kp kc o", kp=128),
                )
                out_sb = acts.tile([128, CH, S], out_dt or WEIGHT_DT, tag=out_tag)
                for o in range(CH):
                    ps = mm_psum.tile([128, S], F32, tag="mm")
                    for k in range(CH):
                        nc.tensor.matmul(
                            ps,
                            lhsT=w_sb[:, k, o * 128:(o + 1) * 128],
                            rhs=rhs_sb[:, k, :],
                            start=(k == 0),
                            stop=(k == CH - 1),
                        )
                    nc.scalar.activation(
                        out=out_sb[:, o, :], in_=ps, func=func,
                        bias=bias_sb[b_name][:, o:o + 1], scale=1.0,
                    )
                return out_sb

            fr_rhs = load_acts(fr_in_T)
            ho_rhs = load_acts(ho_in_T)
            fr_T = dense_T("W_fr_T", "b_fr", fr_rhs, ACTF.Relu, "fr_T")
            fre_T = dense_T("W_fre_T", "b_fre", fr_T, ACTF.Identity, "fre_T")
            ho_T = dense_T("W_ho_T", "b_ho", ho_rhs, ACTF.Tanh, "ho_T")
            hoe_T = dense_T("W_hoe_T", "b_hoe", ho_T, ACTF.Identity, "hoe_T")

            # --- naturalize fr, fr_e, ho_e to [S, D] rows ------------------
            def naturalize(src_sb, tag, dt):
                nat = acts.tile([S, CH, 128], dt, tag=tag)
                for c in range(CH):
                    ps = tr_psum.tile([S, 128], src_sb.dtype, tag="tr")
                    nc.tensor.transpose(ps, src_sb[:, c, :], get_ident(src_sb.dtype))
                    nc.scalar.activation(out=nat[:, c, :], in_=ps, func=ACTF.Copy)
                return nat

            if stage < 1:
                nc.sync.dma_start(
                    out=out_dram[:, :].rearrange("(c p) b -> p c b", p=128),
                    in_=ho_T,
                )
                return nc

            fr_nat = naturalize(fr_T, "fr_nat", STREAM_DT)
            fre_nat = naturalize(fre_T, "fre_nat", STREAM_DT)
            hoe_nat = naturalize(hoe_T, "hoe_nat", STREAM_DT)

            def flat(nat_t, rows=S):
                return nat_t[0:rows, :, :].rearrange("b c p -> b (c p)")

            # --- scores ----------------------------------------------------
            # sc_cols[p, c, b] = score for l = 1 + c*98 + p; sc0 = l=0 row.
            sc_cols = singles.tile([128, 2, S], F32)
            nc.vector.memset(sc_cols[96:128, :, :], NEG)
            sc0 = singles.tile([S, 1], F32)

            # l=0 slot, all batches at once: tanh(fr_e + ho_e) . W_a
            sum0 = singles.tile([S, D], STREAM_DT)
            nc.vector.tensor_add(sum0, flat(fre_nat), flat(hoe_nat))
            ha0 = singles.tile([S, D], STREAM_DT)
            nc.scalar.activation(out=ha0, in_=sum0, func=ACTF.Tanh)
            prod0 = singles.tile([S, D], STREAM_DT)
            nc.vector.tensor_mul(prod0, ha0, wa_bc[0:S, 0, :])
            nc.vector.tensor_reduce(
                out=sc0, in_=prod0, axis=mybir.AxisListType.X, op=ALU.add
            )

            cfe_v = cfe[:, :, :].rearrange("b (c p) d -> b p c d", p=LC)
            conv_v = conv[:, :, :].rearrange("b (c p) d -> b p c d", p=LC)
            conv_tiles = {}

            for b in (range(S) if stage >= 2 else []):
                # prefetch the conv value tile for the later visAtt pass on
                # the second HWDGE ring so it never queues behind cfe loads
                conv_t = convp.tile([LC, 2, D], STREAM_DT, tag="conv")
                nc.scalar.dma_start(out=conv_t, in_=conv_v[b])
                conv_tiles[b] = conv_t

                # ho_e[b, :] broadcast: SBUF-to-SBUF DMA stages the row twice
                # at partition 0 (engines cannot start at partition b), then
                # the otherwise-idle GpSimd fans it to 98 partitions.
                hoe_row = bcast.tile([1, 2, D], STREAM_DT, tag="hoe_row")
                for c in range(2):
                    nc.sync.dma_start(
                        out=hoe_row[:, c, :], in_=flat(hoe_nat)[b:b + 1, :]
                    )
                hoe_bc = bcast.tile([LC, 2, D], STREAM_DT, tag="hoe_bc")
                nc.gpsimd.partition_broadcast(
                    hoe_bc.rearrange("p c d -> p (c d)"),
                    hoe_row[0:1, :, :].rearrange("p c d -> p (c d)"),
                )

                cfe_t = stream.tile([LC, 2, D], STREAM_DT, tag="cfe")
                nc.sync.dma_start(out=cfe_t, in_=cfe_v[b])
                sum_t = stream.tile([LC, 2, D], STREAM_DT, tag="sum", bufs=2)
                nc.vector.tensor_add(
                    sum_t.rearrange("p c d -> p (c d)"),
                    cfe_t.rearrange("p c d -> p (c d)"),
                    hoe_bc.rearrange("p c d -> p (c d)"),
                )
                ha_t = stream.tile([LC, 2, D], STREAM_DT, tag="ha", bufs=2)
                nc.scalar.activation(
                    out=ha_t.rearrange("p c d -> p (c d)"),
                    in_=sum_t.rearrange("p c d -> p (c d)"),
                    func=ACTF.Tanh,
                )
                prod = stream.tile([LC, 2, D], STREAM_DT, tag="prod", bufs=2)
                nc.vector.tensor_mul(
                    prod.rearrange("p c d -> p (c d)"),
                    ha_t.rearrange("p c d -> p (c d)"),
                    wa_bc[0:LC, :, :].rearrange("p c d -> p (c d)"),
                )
                nc.vector.tensor_reduce(
                    out=sc_cols[0:LC, :, b], in_=prod,
                    axis=mybir.AxisListType.X, op=ALU.add,
                )

            if stage < 3:
                nc.sync.dma_start(
                    out=out_dram[:, :].rearrange("(c p) b -> p c b", p=128),
                    in_=ho_T,
                )
                return nc

            # --- softmax over l=0..196 (batched, [S, *] layout) ------------
            sc_nat = singles.tile([S, 2, 128], F32)
            for c in range(2):
                ps = tr_psum.tile([S, 128], F32, tag="tr")
                nc.tensor.transpose(ps, sc_cols[:, c, :], get_ident(F32))
                nc.scalar.activation(out=sc_nat[:, c, :], in_=ps, func=ACTF.Copy)

            neg_mx = singles.tile([S, 1], F32)
            nc.vector.tensor_reduce(
                out=neg_mx, in_=sc_nat.rearrange("p a b -> p (a b)"),
                axis=mybir.AxisListType.X, op=ALU.max, negate=True,
            )
            neg_sc0 = singles.tile([S, 1], F32)
            nc.vector.tensor_scalar_mul(neg_sc0, sc0, -1.0)
            nc.vector.tensor_tensor(neg_mx, neg_mx, neg_sc0, op=ALU.min)

            exp_t = singles.tile([S, 2, 128], F32)
            nc.scalar.activation(
                out=exp_t.rearrange("p a b -> p (a b)"),
                in_=sc_nat.rearrange("p a b -> p (a b)"),
                func=ACTF.Exp, bias=neg_mx, scale=1.0,
            )
            exp0 = singles.tile([S, 1], F32)
            nc.scalar.activation(out=exp0, in_=sc0, func=ACTF.Exp,
                                 bias=neg_mx, scale=1.0)
            ssum = singles.tile([S, 1], F32)
            nc.vector.tensor_reduce(
                out=ssum, in_=exp_t.rearrange("p a b -> p (a b)"),
                axis=mybir.AxisListType.X, op=ALU.add,
            )
            nc.vector.tensor_add(ssum, ssum, exp0)
            rsum = singles.tile([S, 1], F32)
            nc.vector.reciprocal(rsum, ssum)

            pi_nat = singles.tile([S, 2, 128], STREAM_DT)
            nc.vector.tensor_scalar_mul(
                pi_nat.rearrange("p a b -> p (a b)"),
                exp_t.rearrange("p a b -> p (a b)"),
                rsum,
            )
            pi0 = singles.tile([S, 1], F32)
            nc.vector.tensor_tensor(pi0, exp0, rsum, op=ALU.mult)
            pi_cols = singles.tile([128, 2, S], STREAM_DT)
            for c in range(2):
                ps = tr_psum.tile([128, S], STREAM_DT, tag="tr")
                nc.tensor.transpose(
                    ps, pi_nat[:, c, :], get_ident(STREAM_DT)[:S, :S]
                )
                nc.scalar.activation(out=pi_cols[:, c, :], in_=ps, func=ACTF.Copy)

            # l=0 visAtt term: PI[b,0] * fr[b,:], transposed into [d, b]
            va0_nat = singles.tile([S, D], STREAM_DT)
            nc.vector.tensor_scalar_mul(va0_nat, flat(fr_nat), pi0)
            va0_T = acts.tile([128, CH, S], F32, tag="va0_T")
            for c in range(CH):
                ps = tr_psum.tile([128, S], STREAM_DT, tag="tr")
                nc.tensor.transpose(
                    ps, va0_nat[:, c * 128:(c + 1) * 128],
                    get_ident(STREAM_DT)[:S, :S],
                )
                nc.scalar.activation(out=va0_T[:, c, :], in_=ps, func=ACTF.Copy)

            if stage < 4:
                nc.sync.dma_start(
                    out=out_dram[:, :].rearrange("(c p) b -> p c b", p=128),
                    in_=ho_T,
                )
                return nc

            # --- visAtt: accumulate conv chunks into [d, b] PSUM -----------
            # One pending accumulation group per PSUM zero region: each
            # (b, s_) pair's start/stop matmuls are issued back-to-back.
            va = va_psum.tile([128, CH, S], F32)
            for b in range(S):
                conv_t = conv_tiles[b]
                for s_ in range(CH):
                    for c in range(2):
                        nc.tensor.matmul(
                            va[:, s_, b:b + 1],
                            lhsT=conv_t[:, c, s_ * 128:(s_ + 1) * 128],
                            rhs=pi_cols[0:LC, c, b:b + 1],
                            start=(c == 0),
                            stop=(c == 1),
                        )

            # --- atten_out = visAtt + va0 + ho; h = tanh(W_h @ .) ----------
            attn = acts.tile([128, CH, S], WEIGHT_DT, tag="attn")
            nc.vector.tensor_add(attn, va, ho_T)
            nc.vector.tensor_add(attn, attn, va0_T)

            h_sb = dense_T("W_h_T", "b_h", attn, ACTF.Tanh, "h", out_dt=F32)
            nc.sync.dma_start(
                out=out_dram[:, :].rearrange("(c p) b -> p c b", p=128), in_=h_sb
            )

    return nc


_NC_CACHE = {}


def _get_nc(stage: int = 9):
    key = ("nc", stage)
    if key not in _NC_CACHE:
        nc = build_nc(stage)
        nc.compile()
        _NC_CACHE[key] = nc
    return _NC_CACHE[key]


def make_in_maps(inputs):
    sdt = _np_dt(STREAM_DT)
    wdt = _np_dt(WEIGHT_DT)
    shared = {}
    for wn in ("W_fr", "W_fre", "W_ho", "W_hoe", "W_h"):
        shared[wn + "_T"] = np.ascontiguousarray(inputs[wn].T.astype(wdt))
    for bn in ("b_fr", "b_fre", "b_ho", "b_hoe", "b_h"):
        shared[bn] = np.ascontiguousarray(inputs[bn].astype(np.float32))
    shared["W_a"] = np.ascontiguousarray(
        inputs["W_a"].reshape(-1).astype(sdt)
    )
    in_maps = []
    for i in range(N_CORES):
        sl = slice(i * S, (i + 1) * S)
        m = dict(shared)
        m["fake_region_T"] = np.ascontiguousarray(
            inputs["fake_region"][sl].T.astype(wdt)
        )
        m["h_out_T"] = np.ascontiguousarray(inputs["h_out"][sl].T.astype(wdt))
        m["conv_feat"] = np.ascontiguousarray(inputs["conv_feat"][sl].astype(sdt))
        m["conv_feat_embed"] = np.ascontiguousarray(
            inputs["conv_feat_embed"][sl].astype(sdt)
        )
        in_maps.append(m)
    return in_maps


def run(inputs, trace=False, trace_kwargs=None, stage=9):
    nc = _get_nc(stage)
    in_maps = make_in_maps(inputs)
    res = run_bass_kernel_spmd(
        nc, in_maps, core_ids=list(range(N_CORES)), trace=trace,
        **(trace_kwargs or {}),
    )
    shards = [res.results[i]["out"] for i in range(N_CORES)]
    h = np.concatenate([s.T for s in shards], axis=0).astype(np.float32)
    return h, res


def kernel(**inputs) -> np.ndarray:
    h, _ = run(inputs, trace=False)
    return h


if __name__ == "__main__":
    nc = build_nc()
    print(f"built ok: {len(nc.inst_map)} instructions")


# revision 47
# speedup vs baseline: 1.1632x; 1.1632x over previous
"""AdaAtt attention block on 8 TRN2 NeuronCores.

Sharding: data-parallel over batch (16 batches/core), weights replicated.
All dense-layer activations are kept in transposed layout [D_part, batch]
so every D x D matmul consumes host-pre-transposed weights with natural
DMA, and bias+nonlinearity fuse into one ScalarE op out of PSUM.

The conv streams move as one [98, 2, 1024] DMA per batch (two 98-row
l-chunks) — multiple 4KB rows per partition makes the descriptors fan
across SDMA engines (~205 GB/s vs 33 GB/s for single-row-per-partition).
The fake-region slot (l=0) is handled by a separate batched [16, 1024]
pipeline instead of per-batch row injection.

Score pass per (batch, chunk): DVE add (ho_e broadcast via PE outer
product into PSUM) -> ACT tanh -> DVE scalar_tensor_tensor (x*W_a fused
with the free-axis sum) -> score columns. One batched softmax for all 16
batches (b_a dropped: softmax is shift-invariant). visAtt accumulates
into [d, batch] PSUM via per-chunk matvecs; the l=0 term joins through
PE transposes of PI0*fr. Then +ho_T and the final tanh(W_h @ .) matmul.
"""

import sys

if "/opt/trn_rl_repo" not in sys.path:
    sys.path.insert(0, "/opt/trn_rl_repo")

import numpy as np

import concourse.bass as bass
import concourse.tile as tile
from concourse import mybir
from concourse import bacc
from concourse.bass_utils import run_bass_kernel_spmd
from concourse.masks import make_identity

# ---------------------------------------------------------------------------

B, L, D = 128, 196, 1024
N_CORES = 8
S = B // N_CORES          # batches per core
CH = D // 128             # 128-wide chunks of D
LC = L // 2               # l-chunk rows (98); l=0 slot handled separately
NEG = -1.0e30

F32 = mybir.dt.float32
BF16 = mybir.dt.bfloat16
STREAM_DT = BF16          # dtype of conv_feat/conv_feat_embed stream
WEIGHT_DT = BF16          # dtype of the five D x D weights / dense math

ACTF = mybir.ActivationFunctionType
ALU = mybir.AluOpType


def _np_dt(dt):
    if dt == F32:
        return np.float32
    import ml_dtypes

    return ml_dtypes.bfloat16


def build_nc(stage: int = 9) -> bass.Bass:
    # stage: debug bisection knob — 0: dense chains, 1: +naturalize,
    # 2: +score pass, 3: +softmax, 4+: full kernel.
    nc = bacc.Bacc()

    def param(name, shape, dt=F32, out=False):
        return nc.declare_dram_parameter(name, list(shape), dt, isOutput=out)

    fr_in_T = param("fake_region_T", (D, S), WEIGHT_DT)
    ho_in_T = param("h_out_T", (D, S), WEIGHT_DT)
    w_names = ("W_fr_T", "W_fre_T", "W_ho_T", "W_hoe_T", "W_h_T")
    w_dram = {n: param(n, (D, D), WEIGHT_DT) for n in w_names}
    b_names = ("b_fr", "b_fre", "b_ho", "b_hoe", "b_h")
    b_dram = {n: param(n, (D,)) for n in b_names}
    wa_dram = param("W_a", (D,), STREAM_DT)
    conv = param("conv_feat", (S, L, D), STREAM_DT)
    cfe = param("conv_feat_embed", (S, L, D), STREAM_DT)
    out_dram = param("out", (D, S), out=True)

    with tile.TileContext(nc) as tc:
        with (
            tc.tile_pool(name="singles", bufs=1) as singles,
            tc.tile_pool(name="wpool", bufs=2) as wpool,
            tc.tile_pool(name="acts", bufs=1) as acts,
            tc.tile_pool(name="stream", bufs=3) as stream,
            tc.tile_pool(name="convp", bufs=S) as convp,
            tc.tile_pool(name="bcast", bufs=2) as bcast,
            tc.tile_pool(name="mm_psum", bufs=2, space="PSUM") as mm_psum,
            tc.tile_pool(name="tr_psum", bufs=2, space="PSUM") as tr_psum,
            tc.tile_pool(name="va_psum", bufs=1, space="PSUM") as va_psum,
        ):
            # --- constants -------------------------------------------------
            _idents = {}

            def get_ident(dt):
                if dt not in _idents:
                    t = singles.tile([128, 128], dt, tag=f"ident_{dt}")
                    make_identity(nc, t)
                    _idents[dt] = t
                return _idents[dt]

            wa_bc = singles.tile([128, D], STREAM_DT)
            wa_ap = wa_dram[:]
            nc.sync.dma_start(
                out=wa_bc,
                in_=bass.AP(
                    tensor=wa_ap.tensor, offset=wa_ap.offset,
                    ap=[[0, 128]] + list(wa_ap.ap),
                ),
            )

            bias_sb = {}
            for n in b_names:
                t = singles.tile([128, CH], F32, tag=f"bias_{n}")
                nc.sync.dma_start(
                    out=t, in_=b_dram[n][:].rearrange("(c p) -> p c", p=128)
                )
                bias_sb[n] = t

            # --- stage A: fr/fr_e/ho/ho_e in transposed layout -------------
            def load_acts(src):
                t = acts.tile([128, CH, S], WEIGHT_DT, tag="acts_in")
                nc.sync.dma_start(
                    out=t, in_=src[:, :].rearrange("(c p) b -> p c b", p=128)
                )
                return t

            _w_ring = [0]

            def dense_T(w_name, b_name, rhs_sb, func, out_tag, out_dt=None):
                """out[128, CH, S] = func(W^T . rhs + b), transposed layout."""
                w_sb = wpool.tile([128, CH, D], WEIGHT_DT, tag="w")
                # alternate HWDGE rings so two weight loads stream in parallel
                ring = nc.sync if _w_ring[0] % 2 == 0 else nc.scalar
                _w_ring[0] += 1
                ring.dma_start(
                    out=w_sb,
                    in_=w_dram[w_name][:, :].rearrange("(kc kp) o -> kp kc o", kp=128),
                )
                out_sb = acts.tile([128, CH, S], out_dt or WEIGHT_DT, tag=out_tag)
                for o in range(CH):
                    ps = mm_psum.tile([128, S], F32, tag="mm")
                    for k in range(CH):
                        nc.tensor.matmul(
                            ps,
                            lhsT=w_sb[:, k, o * 128:(o + 1) * 128],
                            rhs=rhs_sb[:, k, :],
                            start=(k == 0),
                            stop=(k == CH - 1),
                        )
                    nc.scalar.activation(
                        out=out_sb[:, o, :], in_=ps, func=func,
                        bias=bias_sb[b_name][:, o:o + 1], scale=1.0,
                    )
                return out_sb

            fr_rhs = load_acts(fr_in_T)
            ho_rhs = load_acts(ho_in_T)
            fr_T = dense_T("W_fr_T", "b_fr", fr_rhs, ACTF.Relu, "fr_T")
            fre_T = dense_T("W_fre_T", "b_fre", fr_T, ACTF.Identity, "fre_T")
            ho_T = dense_T("W_ho_T", "b_ho", ho_rhs, ACTF.Tanh, "ho_T")
            hoe_T = dense_T("W_hoe_T", "b_hoe", ho_T, ACTF.Identity, "hoe_T")

            # --- naturalize fr, fr_e, ho_e to [S, D] rows ------------------
            def naturalize(src_sb, tag, dt):
                nat = acts.tile([S, CH, 128], dt, tag=tag)
                for c in range(CH):
                    ps = tr_psum.tile([S, 128], src_sb.dtype, tag="tr")
                    nc.tensor.transpose(ps, src_sb[:, c, :], get_ident(src_sb.dtype))
                    nc.scalar.activation(out=nat[:, c, :], in_=ps, func=ACTF.Copy)
                return nat

            if stage < 1:
                nc.sync.dma_start(
                    out=out_dram[:, :].rearrange("(c p) b -> p c b", p=128),
                    in_=ho_T,
                )
                return nc

            fr_nat = naturalize(fr_T, "fr_nat", STREAM_DT)
            fre_nat = naturalize(fre_T, "fre_nat", STREAM_DT)
            hoe_nat = naturalize(hoe_T, "hoe_nat", STREAM_DT)

            def flat(nat_t, rows=S):
                return nat_t[0:rows, :, :].rearrange("b c p -> b (c p)")

            # --- scores ----------------------------------------------------
            # sc_cols[p, c, b] = score for l = 1 + c*98 + p; sc0 = l=0 row.
            sc_cols = singles.tile([128, 2, S], F32)
            nc.vector.memset(sc_cols[96:128, :, :], NEG)
            sc0 = singles.tile([S, 1], F32)

            # l=0 slot, all batches at once: tanh(fr_e + ho_e) . W_a
            sum0 = singles.tile([S, D], STREAM_DT)
            nc.vector.tensor_add(sum0, flat(fre_nat), flat(hoe_nat))
            ha0 = singles.tile([S, D], STREAM_DT)
            nc.scalar.activation(out=ha0, in_=sum0, func=ACTF.Tanh)
            junk0 = singles.tile([S, D], STREAM_DT)
            nc.vector.scalar_tensor_tensor(
                out=junk0, in0=ha0, scalar=1.0, in1=wa_bc[0:S, :],
                op0=ALU.mult, op1=ALU.mult, accum_out=sc0,
            )

            cfe_v = cfe[:, :, :].rearrange("b (c p) d -> b p c d", p=LC)
            conv_v = conv[:, :, :].rearrange("b (c p) d -> b p c d", p=LC)
            conv_tiles = {}

            for b in (range(S) if stage >= 2 else []):
                # ho_e[b, :] broadcast: SBUF-to-SBUF DMA stages the row at
                # partition 0 (engines cannot start at partition b), then the
                # otherwise-idle GpSimd fans it to 128 partitions.
                hoe_row = bcast.tile([1, D], STREAM_DT, tag="hoe_row")
                nc.sync.dma_start(out=hoe_row, in_=flat(hoe_nat)[b:b + 1, :])
                hoe_bc = bcast.tile([128, D], STREAM_DT, tag="hoe_bc")
                nc.gpsimd.partition_broadcast(hoe_bc, hoe_row[0:1, :])

                cfe_t = stream.tile([LC, 2, D], STREAM_DT, tag="cfe")
                nc.sync.dma_start(out=cfe_t, in_=cfe_v[b])
                # prefetch the conv value tile for the later visAtt pass;
                # the ring keeps up with the DVE-paced score chain
                conv_t = convp.tile([LC, 2, D], STREAM_DT, tag="conv")
                nc.sync.dma_start(out=conv_t, in_=conv_v[b])
                conv_tiles[b] = conv_t

                for c in range(2):
                    sum_t = stream.tile([LC, D], STREAM_DT, tag="sum", bufs=2)
                    nc.vector.tensor_add(sum_t, cfe_t[:, c, :], hoe_bc[0:LC, :])
                    ha_t = stream.tile([LC, D], STREAM_DT, tag="ha", bufs=2)
                    nc.scalar.activation(out=ha_t, in_=sum_t, func=ACTF.Tanh)
                    junk = stream.tile([LC, D], STREAM_DT, tag="junk", bufs=2)
                    nc.vector.scalar_tensor_tensor(
                        out=junk, in0=ha_t, scalar=1.0,
                        in1=wa_bc[0:LC, :], op0=ALU.mult, op1=ALU.mult,
                        accum_out=sc_cols[0:LC, c, b:b + 1],
                    )

            if stage < 3:
                nc.sync.dma_start(
                    out=out_dram[:, :].rearrange("(c p) b -> p c b", p=128),
                    in_=ho_T,
                )
                return nc

            # --- softmax over l=0..196 (batched, [S, *] layout) ------------
            sc_nat = singles.tile([S, 2, 128], F32)
            for c in range(2):
                ps = tr_psum.tile([S, 128], F32, tag="tr")
                nc.tensor.transpose(ps, sc_cols[:, c, :], get_ident(F32))
                nc.scalar.activation(out=sc_nat[:, c, :], in_=ps, func=ACTF.Copy)

            neg_mx = singles.tile([S, 1], F32)
            nc.vector.tensor_reduce(
                out=neg_mx, in_=sc_nat.rearrange("p a b -> p (a b)"),
                axis=mybir.AxisListType.X, op=ALU.max, negate=True,
            )
            neg_sc0 = singles.tile([S, 1], F32)
            nc.vector.tensor_scalar_mul(neg_sc0, sc0, -1.0)
            nc.vector.tensor_tensor(neg_mx, neg_mx, neg_sc0, op=ALU.min)

            exp_t = singles.tile([S, 2, 128], F32)
            nc.scalar.activation(
                out=exp_t.rearrange("p a b -> p (a b)"),
                in_=sc_nat.rearrange("p a b -> p (a b)"),
                func=ACTF.Exp, bias=neg_mx, scale=1.0,
            )
            exp0 = singles.tile([S, 1], F32)
            nc.scalar.activation(out=exp0, in_=sc0, func=ACTF.Exp,
                                 bias=neg_mx, scale=1.0)
            ssum = singles.tile([S, 1], F32)
            nc.vector.tensor_reduce(
                out=ssum, in_=exp_t.rearrange("p a b -> p (a b)"),
                axis=mybir.AxisListType.X, op=ALU.add,
            )
            nc.vector.tensor_add(ssum, ssum, exp0)
            rsum = singles.tile([S, 1], F32)
            nc.vector.reciprocal(rsum, ssum)

            pi_nat = singles.tile([S, 2, 128], STREAM_DT)
            nc.vector.tensor_scalar_mul(
                pi_nat.rearrange("p a b -> p (a b)"),
                exp_t.rearrange("p a b -> p (a b)"),
                rsum,
            )
            pi0 = singles.tile([S, 1], F32)
            nc.vector.tensor_tensor(pi0, exp0, rsum, op=ALU.mult)
            pi_cols = singles.tile([128, 2, S], STREAM_DT)
            for c in range(2):
                ps = tr_psum.tile([128, S], STREAM_DT, tag="tr")
                nc.tensor.transpose(
                    ps, pi_nat[:, c, :], get_ident(STREAM_DT)[:S, :S]
                )
                nc.scalar.activation(out=pi_cols[:, c, :], in_=ps, func=ACTF.Copy)

            # l=0 visAtt term: PI[b,0] * fr[b,:], transposed into [d, b]
            va0_nat = singles.tile([S, D], STREAM_DT)
            nc.vector.tensor_scalar_mul(va0_nat, flat(fr_nat), pi0)
            va0_T = acts.tile([128, CH, S], F32, tag="va0_T")
            for c in range(CH):
                ps = tr_psum.tile([128, S], STREAM_DT, tag="tr")
                nc.tensor.transpose(
                    ps, va0_nat[:, c * 128:(c + 1) * 128],
                    get_ident(STREAM_DT)[:S, :S],
                )
                nc.scalar.activation(out=va0_T[:, c, :], in_=ps, func=ACTF.Copy)

            if stage < 4:
                nc.sync.dma_start(
                    out=out_dram[:, :].rearrange("(c p) b -> p c b", p=128),
                    in_=ho_T,
                )
                return nc

            # --- visAtt: accumulate conv chunks into [d, b] PSUM -----------
            # One pending accumulation group per PSUM zero region: each
            # (b, s_) pair's start/stop matmuls are issued back-to-back.
            va = va_psum.tile([128, CH, S], F32)
            for b in range(S):
                conv_t = conv_tiles[b]
                for s_ in range(CH):
                    for c in range(2):
                        nc.tensor.matmul(
                            va[:, s_, b:b + 1],
                            lhsT=conv_t[:, c, s_ * 128:(s_ + 1) * 128],
                            rhs=pi_cols[0:LC, c, b:b + 1],
                            start=(c == 0),
                            stop=(c == 1),
                        )

            # --- atten_out = visAtt + va0 + ho; h = tanh(W_h @ .) ----------
            attn = acts.tile([128, CH, S], WEIGHT_DT, tag="attn")
            nc.vector.tensor_add(attn, va, ho_T)
            nc.vector.tensor_add(attn, attn, va0_T)

            h_sb = dense_T("W_h_T", "b_h", attn, ACTF.Tanh, "h", out_dt=F32)
            nc.sync.dma_start(
                out=out_dram[:, :].rearrange("(c p) b -> p c b", p=128), in_=h_sb
            )

    return nc


_NC_CACHE = {}


def _get_nc(stage: int = 9):
    key = ("nc", stage)
    if key not in _NC_CACHE:
        nc = build_nc(stage)
        nc.compile()
        _NC_CACHE[key] = nc
    return _NC_CACHE[key]


def make_in_maps(inputs):
    sdt = _np_dt(STREAM_DT)
    wdt = _np_dt(WEIGHT_DT)
    shared = {}
    for wn in ("W_fr", "W_fre", "W_ho", "W_hoe", "W_h"):
        shared[wn + "_T"] = np.ascontiguousarray(inputs[wn].T.astype(wdt))
    for bn in ("b_fr", "b_fre", "b_ho", "b_hoe", "b_h"):
        shared[bn] = np.ascontiguousarray(inputs[bn].astype(np.float32))
    shared["W_a"] = np.ascontiguousarray(
        inputs["W_a"].reshape(-1).astype(sdt)
    )
    in_maps = []
    for i in range(N_CORES):
        sl = slice(i * S, (i + 1) * S)
        m = dict(shared)
        m["fake_region_T"] = np.ascontiguousarray(
            inputs["fake_region"][sl].T.astype(wdt)
        )
        m["h_out_T"] = np.ascontiguousarray(inputs["h_out"][sl].T.astype(wdt))
        m["conv_feat"] = np.ascontiguousarray(inputs["conv_feat"][sl].astype(sdt))
        m["conv_feat_embed"] = np.ascontiguousarray(
            inputs["conv_feat_embed"][sl].astype(sdt)
        )
        in_maps.append(m)
    return in_maps


def run(inputs, trace=False, trace_kwargs=None, stage=9):
    nc = _get_nc(stage)
    in_maps = make_in_maps(inputs)
    res = run_bass_kernel_spmd(
        nc, in_maps, core_ids=list(range(N_CORES)), trace=trace,
        **(trace_kwargs or {}),
    )
    shards = [res.results[i]["out"] for i in range(N_CORES)]
    h = np.concatenate([s.T for s in shards], axis=0).astype(np.float32)
    return h, res


def kernel(**inputs) -> np.ndarray:
    h, _ = run(inputs, trace=False)
    return h


if __name__ == "__main__":
    nc = build_nc()
    print(f"built ok: {len(nc.inst_map)} instructions")


# revision 51
# speedup vs baseline: 1.5778x; 1.3564x over previous
"""AdaAtt attention block on 8 TRN2 NeuronCores.

Sharding: data-parallel over batch (16 batches/core), weights replicated.
All dense-layer activations are kept in transposed layout [D_part, batch]
so every D x D matmul consumes host-pre-transposed weights with natural
DMA, and bias+nonlinearity fuse into one ScalarE op out of PSUM.

The conv streams move as one [98, 2, 1024] DMA per batch (two 98-row
l-chunks) — multiple 4KB rows per partition makes the descriptors fan
across SDMA engines (~205 GB/s vs 33 GB/s for single-row-per-partition).
The fake-region slot (l=0) is handled by a separate batched [16, 1024]
pipeline instead of per-batch row injection.

Score pass per (batch, chunk): DVE add (ho_e broadcast via PE outer
product into PSUM) -> ACT tanh -> DVE scalar_tensor_tensor (x*W_a fused
with the free-axis sum) -> score columns. One batched softmax for all 16
batches (b_a dropped: softmax is shift-invariant). visAtt accumulates
into [d, batch] PSUM via per-chunk matvecs; the l=0 term joins through
PE transposes of PI0*fr. Then +ho_T and the final tanh(W_h @ .) matmul.
"""

import sys

if "/opt/trn_rl_repo" not in sys.path:
    sys.path.insert(0, "/opt/trn_rl_repo")

import numpy as np

import concourse.bass as bass
import concourse.tile as tile
from concourse import mybir
from concourse import bacc
from concourse.bass_utils import run_bass_kernel_spmd
from concourse.masks import make_identity

# ---------------------------------------------------------------------------

B, L, D = 128, 196, 1024
N_CORES = 8
S = B // N_CORES          # batches per core
CH = D // 128             # 128-wide chunks of D
LC = L // 2               # l-chunk rows (98); l=0 slot handled separately
NEG = -1.0e30

F32 = mybir.dt.float32
BF16 = mybir.dt.bfloat16
STREAM_DT = BF16          # dtype of conv_feat/conv_feat_embed stream
WEIGHT_DT = BF16          # dtype of the five D x D weights / dense math

ACTF = mybir.ActivationFunctionType
ALU = mybir.AluOpType


def _np_dt(dt):
    if dt == F32:
        return np.float32
    import ml_dtypes

    return ml_dtypes.bfloat16


def build_nc(stage: int = 9) -> bass.Bass:
    # stage: debug bisection knob — 0: dense chains, 1: +naturalize,
    # 2: +score pass, 3: +softmax, 4+: full kernel.
    nc = bacc.Bacc()

    def param(name, shape, dt=F32, out=False):
        return nc.declare_dram_parameter(name, list(shape), dt, isOutput=out)

    fr_in_T = param("fake_region_T", (D, S), WEIGHT_DT)
    ho_in_T = param("h_out_T", (D, S), WEIGHT_DT)
    w_names = ("W_fr_T", "W_fre_T", "W_ho_T", "W_hoe_T", "W_h_T")
    w_dram = {n: param(n, (D, D), WEIGHT_DT) for n in w_names}
    b_names = ("b_fr", "b_fre", "b_ho", "b_hoe", "b_h")
    b_dram = {n: param(n, (D,)) for n in b_names}
    wa_dram = param("W_a", (D,), STREAM_DT)
    conv = param("conv_feat", (S, L, D), STREAM_DT)
    cfe_T = param("conv_feat_embed_T", (S, D, L), STREAM_DT)
    out_dram = param("out", (D, S), out=True)

    with tile.TileContext(nc) as tc:
        with (
            tc.tile_pool(name="singles", bufs=1) as singles,
            tc.tile_pool(name="wpool", bufs=2) as wpool,
            tc.tile_pool(name="acts", bufs=1) as acts,
            tc.tile_pool(name="stream", bufs=3) as stream,
            tc.tile_pool(name="convp", bufs=S) as convp,
            tc.tile_pool(name="bcast", bufs=2) as bcast,
            tc.tile_pool(name="mm_psum", bufs=2, space="PSUM") as mm_psum,
            tc.tile_pool(name="tr_psum", bufs=2, space="PSUM") as tr_psum,
            tc.tile_pool(name="va_psum", bufs=1, space="PSUM") as va_psum,
        ):
            # --- constants -------------------------------------------------
            _idents = {}

            def get_ident(dt):
                if dt not in _idents:
                    t = singles.tile([128, 128], dt, tag=f"ident_{dt}")
                    make_identity(nc, t)
                    _idents[dt] = t
                return _idents[dt]

            wa_bc = singles.tile([128, D], STREAM_DT)
            wa_ap = wa_dram[:]
            nc.sync.dma_start(
                out=wa_bc,
                in_=bass.AP(
                    tensor=wa_ap.tensor, offset=wa_ap.offset,
                    ap=[[0, 128]] + list(wa_ap.ap),
                ),
            )

            bias_sb = {}
            for n in b_names:
                t = singles.tile([128, CH], F32, tag=f"bias_{n}")
                nc.sync.dma_start(
                    out=t, in_=b_dram[n][:].rearrange("(c p) -> p c", p=128)
                )
                bias_sb[n] = t

            # --- stage A: fr/fr_e/ho/ho_e in transposed layout -------------
            def load_acts(src):
                t = acts.tile([128, CH, S], WEIGHT_DT, tag="acts_in")
                nc.sync.dma_start(
                    out=t, in_=src[:, :].rearrange("(c p) b -> p c b", p=128)
                )
                return t

            _w_ring = [0]

            def dense_T(w_name, b_name, rhs_sb, func, out_tag, out_dt=None):
                """out[128, CH, S] = func(W^T . rhs + b), transposed layout."""
                w_sb = wpool.tile([128, CH, D], WEIGHT_DT, tag="w")
                # alternate HWDGE rings so two weight loads stream in parallel
                ring = nc.sync if _w_ring[0] % 2 == 0 else nc.scalar
                _w_ring[0] += 1
                ring.dma_start(
                    out=w_sb,
                    in_=w_dram[w_name][:, :].rearrange("(kc kp) o -> kp kc o", kp=128),
                )
                out_sb = acts.tile([128, CH, S], out_dt or WEIGHT_DT, tag=out_tag)
                for o in range(CH):
                    ps = mm_psum.tile([128, S], F32, tag="mm")
                    for k in range(CH):
                        nc.tensor.matmul(
                            ps,
                            lhsT=w_sb[:, k, o * 128:(o + 1) * 128],
                            rhs=rhs_sb[:, k, :],
                            start=(k == 0),
                            stop=(k == CH - 1),
                        )
                    nc.scalar.activation(
                        out=out_sb[:, o, :], in_=ps, func=func,
                        bias=bias_sb[b_name][:, o:o + 1], scale=1.0,
                    )
                return out_sb

            fr_rhs = load_acts(fr_in_T)
            ho_rhs = load_acts(ho_in_T)
            fr_T = dense_T("W_fr_T", "b_fr", fr_rhs, ACTF.Relu, "fr_T")
            fre_T = dense_T("W_fre_T", "b_fre", fr_T, ACTF.Identity, "fre_T")
            ho_T = dense_T("W_ho_T", "b_ho", ho_rhs, ACTF.Tanh, "ho_T")
            hoe_T = dense_T("W_hoe_T", "b_hoe", ho_T, ACTF.Identity, "hoe_T",
                            out_dt=F32)

            # --- naturalize fr, fr_e, ho_e to [S, D] rows ------------------
            def naturalize(src_sb, tag, dt):
                nat = acts.tile([S, CH, 128], dt, tag=tag)
                for c in range(CH):
                    ps = tr_psum.tile([S, 128], src_sb.dtype, tag="tr")
                    nc.tensor.transpose(ps, src_sb[:, c, :], get_ident(src_sb.dtype))
                    nc.scalar.activation(out=nat[:, c, :], in_=ps, func=ACTF.Copy)
                return nat

            if stage < 1:
                nc.sync.dma_start(
                    out=out_dram[:, :].rearrange("(c p) b -> p c b", p=128),
                    in_=ho_T,
                )
                return nc

            fr_nat = naturalize(fr_T, "fr_nat", STREAM_DT)
            fre_nat = naturalize(fre_T, "fre_nat", STREAM_DT)
            hoe_nat = naturalize(hoe_T, "hoe_nat", STREAM_DT)

            def flat(nat_t, rows=S):
                return nat_t[0:rows, :, :].rearrange("b c p -> b (c p)")

            # --- scores ----------------------------------------------------
            # sc_cols[p, c, b] = score for l = 1 + c*98 + p; sc0 = l=0 row.
            sc_cols = singles.tile([128, 2, S], F32)
            nc.vector.memset(sc_cols[96:128, :, :], NEG)
            sc0 = singles.tile([S, 1], F32)

            # l=0 slot, all batches at once: tanh(fr_e + ho_e) . W_a
            sum0 = singles.tile([S, D], STREAM_DT)
            nc.vector.tensor_add(sum0, flat(fre_nat), flat(hoe_nat))
            ha0 = singles.tile([S, D], STREAM_DT)
            nc.scalar.activation(out=ha0, in_=sum0, func=ACTF.Tanh)
            junk0 = singles.tile([S, D], STREAM_DT)
            nc.vector.scalar_tensor_tensor(
                out=junk0, in0=ha0, scalar=1.0, in1=wa_bc[0:S, :],
                op0=ALU.mult, op1=ALU.mult, accum_out=sc0,
            )

            # cfe arrives host-transposed [b, d, l]: the ho_e add is then a
            # per-partition tensor_scalar (no broadcast needed), tanh batches
            # into one ACT op per batch, and the W_a reduction becomes PE
            # matvecs accumulating straight into score-column PSUM.
            cfeT_v = cfe_T[:, :, :].rearrange("b (s p) l -> b p s l", p=128)
            conv_v = conv[:, :, :].rearrange("b (c p) d -> b p c d", p=LC)
            conv_tiles = {}

            wa_cols = singles.tile([128, CH], STREAM_DT)
            nc.sync.dma_start(
                out=wa_cols, in_=wa_dram[:].rearrange("(s p) -> p s", p=128)
            )
            sc_ps = va_psum.tile([128, 2, S], F32, tag="sc_ps")

            for b in (range(S) if stage >= 2 else []):
                cfeT_t = stream.tile([128, CH, L], STREAM_DT, tag="cfeT")
                nc.sync.dma_start(out=cfeT_t, in_=cfeT_v[b])
                # prefetch the conv value tile for the later visAtt pass
                conv_t = convp.tile([LC, 2, D], STREAM_DT, tag="conv")
                nc.sync.dma_start(out=conv_t, in_=conv_v[b])
                conv_tiles[b] = conv_t

                sum_T = stream.tile([128, CH, L], STREAM_DT, tag="sumT", bufs=2)
                for s_ in range(CH):
                    nc.vector.tensor_scalar_add(
                        sum_T[:, s_, :], cfeT_t[:, s_, :], hoe_T[:, s_, b:b + 1]
                    )
                ha_T = stream.tile([128, CH, L], STREAM_DT, tag="haT", bufs=2)
                nc.scalar.activation(
                    out=ha_T.rearrange("p s l -> p (s l)"),
                    in_=sum_T.rearrange("p s l -> p (s l)"),
                    func=ACTF.Tanh,
                )
                for c in range(2):
                    for s_ in range(CH):
                        nc.tensor.matmul(
                            sc_ps[0:LC, c, b:b + 1],
                            lhsT=ha_T[:, s_, c * LC:(c + 1) * LC],
                            rhs=wa_cols[:, s_:s_ + 1],
                            start=(s_ == 0),
                            stop=(s_ == CH - 1),
                        )

            nc.scalar.activation(
                out=sc_cols[0:LC, :, :].rearrange("p c b -> p (c b)"),
                in_=sc_ps[0:LC, :, :].rearrange("p c b -> p (c b)"),
                func=ACTF.Copy,
            )

            if stage < 3:
                nc.sync.dma_start(
                    out=out_dram[:, :].rearrange("(c p) b -> p c b", p=128),
                    in_=ho_T,
                )
                return nc

            # --- softmax over l=0..196 (batched, [S, *] layout) ------------
            sc_nat = singles.tile([S, 2, 128], F32)
            for c in range(2):
                ps = tr_psum.tile([S, 128], F32, tag="tr")
                nc.tensor.transpose(ps, sc_cols[:, c, :], get_ident(F32))
                nc.scalar.activation(out=sc_nat[:, c, :], in_=ps, func=ACTF.Copy)

            neg_mx = singles.tile([S, 1], F32)
            nc.vector.tensor_reduce(
                out=neg_mx, in_=sc_nat.rearrange("p a b -> p (a b)"),
                axis=mybir.AxisListType.X, op=ALU.max, negate=True,
            )
            neg_sc0 = singles.tile([S, 1], F32)
            nc.vector.tensor_scalar_mul(neg_sc0, sc0, -1.0)
            nc.vector.tensor_tensor(neg_mx, neg_mx, neg_sc0, op=ALU.min)

            exp_t = singles.tile([S, 2, 128], F32)
            nc.scalar.activation(
                out=exp_t.rearrange("p a b -> p (a b)"),
                in_=sc_nat.rearrange("p a b -> p (a b)"),
                func=ACTF.Exp, bias=neg_mx, scale=1.0,
            )
            exp0 = singles.tile([S, 1], F32)
            nc.scalar.activation(out=exp0, in_=sc0, func=ACTF.Exp,
                                 bias=neg_mx, scale=1.0)
            ssum = singles.tile([S, 1], F32)
            nc.vector.tensor_reduce(
                out=ssum, in_=exp_t.rearrange("p a b -> p (a b)"),
                axis=mybir.AxisListType.X, op=ALU.add,
            )
            nc.vector.tensor_add(ssum, ssum, exp0)
            rsum = singles.tile([S, 1], F32)
            nc.vector.reciprocal(rsum, ssum)

            pi_nat = singles.tile([S, 2, 128], STREAM_DT)
            nc.vector.tensor_scalar_mul(
                pi_nat.rearrange("p a b -> p (a b)"),
                exp_t.rearrange("p a b -> p (a b)"),
                rsum,
            )
            pi0 = singles.tile([S, 1], F32)
            nc.vector.tensor_tensor(pi0, exp0, rsum, op=ALU.mult)
            pi_cols = singles.tile([128, 2, S], STREAM_DT)
            for c in range(2):
                ps = tr_psum.tile([128, S], STREAM_DT, tag="tr")
                nc.tensor.transpose(
                    ps, pi_nat[:, c, :], get_ident(STREAM_DT)[:S, :S]
                )
                nc.scalar.activation(out=pi_cols[:, c, :], in_=ps, func=ACTF.Copy)

            # l=0 visAtt term: PI[b,0] * fr[b,:], transposed into [d, b]
            va0_nat = singles.tile([S, D], STREAM_DT)
            nc.vector.tensor_scalar_mul(va0_nat, flat(fr_nat), pi0)
            va0_T = acts.tile([128, CH, S], F32, tag="va0_T")
            for c in range(CH):
                ps = tr_psum.tile([128, S], STREAM_DT, tag="tr")
                nc.tensor.transpose(
                    ps, va0_nat[:, c * 128:(c + 1) * 128],
                    get_ident(STREAM_DT)[:S, :S],
                )
                nc.scalar.activation(out=va0_T[:, c, :], in_=ps, func=ACTF.Copy)

            if stage < 4:
                nc.sync.dma_start(
                    out=out_dram[:, :].rearrange("(c p) b -> p c b", p=128),
                    in_=ho_T,
                )
                return nc

            # --- visAtt: accumulate conv chunks into [d, b] PSUM -----------
            # One pending accumulation group per PSUM zero region: each
            # (b, s_) pair's start/stop matmuls are issued back-to-back.
            va = va_psum.tile([128, CH, S], F32)
            for b in range(S):
                conv_t = conv_tiles[b]
                for s_ in range(CH):
                    for c in range(2):
                        nc.tensor.matmul(
                            va[:, s_, b:b + 1],
                            lhsT=conv_t[:, c, s_ * 128:(s_ + 1) * 128],
                            rhs=pi_cols[0:LC, c, b:b + 1],
                            start=(c == 0),
                            stop=(c == 1),
                        )

            # --- atten_out = visAtt + va0 + ho; h = tanh(W_h @ .) ----------
            attn = acts.tile([128, CH, S], WEIGHT_DT, tag="attn")
            nc.vector.tensor_add(attn, va, ho_T)
            nc.vector.tensor_add(attn, attn, va0_T)

            h_sb = dense_T("W_h_T", "b_h", attn, ACTF.Tanh, "h", out_dt=F32)
            nc.sync.dma_start(
                out=out_dram[:, :].rearrange("(c p) b -> p c b", p=128), in_=h_sb
            )

    return nc


_NC_CACHE = {}


def _get_nc(stage: int = 9):
    key = ("nc", stage)
    if key not in _NC_CACHE:
        nc = build_nc(stage)
        nc.compile()
        _NC_CACHE[key] = nc
    return _NC_CACHE[key]


def make_in_maps(inputs):
    sdt = _np_dt(STREAM_DT)
    wdt = _np_dt(WEIGHT_DT)
    shared = {}
    for wn in ("W_fr", "W_fre", "W_ho", "W_hoe", "W_h"):
        shared[wn + "_T"] = np.ascontiguousarray(inputs[wn].T.astype(wdt))
    for bn in ("b_fr", "b_fre", "b_ho", "b_hoe", "b_h"):
        shared[bn] = np.ascontiguousarray(inputs[bn].astype(np.float32))
    shared["W_a"] = np.ascontiguousarray(
        inputs["W_a"].reshape(-1).astype(sdt)
    )
    in_maps = []
    for i in range(N_CORES):
        sl = slice(i * S, (i + 1) * S)
        m = dict(shared)
        m["fake_region_T"] = np.ascontiguousarray(
            inputs["fake_region"][sl].T.astype(wdt)
        )
        m["h_out_T"] = np.ascontiguousarray(inputs["h_out"][sl].T.astype(wdt))
        m["conv_feat"] = np.ascontiguousarray(inputs["conv_feat"][sl].astype(sdt))
        m["conv_feat_embed_T"] = np.ascontiguousarray(
            inputs["conv_feat_embed"][sl].transpose(0, 2, 1).astype(sdt)
        )
        in_maps.append(m)
    return in_maps


def run(inputs, trace=False, trace_kwargs=None, stage=9):
    nc = _get_nc(stage)
    in_maps = make_in_maps(inputs)
    res = run_bass_kernel_spmd(
        nc, in_maps, core_ids=list(range(N_CORES)), trace=trace,
        **(trace_kwargs or {}),
    )
    shards = [res.results[i]["out"] for i in range(N_CORES)]
    h = np.concatenate([s.T for s in shards], axis=0).astype(np.float32)
    return h, res


def kernel(**inputs) -> np.ndarray:
    h, _ = run(inputs, trace=False)
    return h


if __name__ == "__main__":
    nc = build_nc()
    print(f"built ok: {len(nc.inst_map)} instructions")
